# revision 12
# baseline (speedup 1.0000x reference)
# Dilated sliding-window attention kernel for 8 Trainium2 NeuronCores.
# Self-contained: hardcodes the problem shapes (B=2, S=2048, D=512, H=8,
# WIN=16, DIL=2, G=64).
#
# Sharding: local-token path is data-parallel over (batch x 4 sequence
# chunks) = 8 cores; each core gets its 496 query tokens plus a
# halo-padded (edge-replicated) 544-token key/value slice, so the
# reference's index clipping is reproduced exactly (including duplicate
# edge keys). The tiny global-token MHA is sharded by head-pairs over
# the 4 chunk-cores of each batch; the out-projection partial sums are
# reduced on the host.
#
# Per-core layout is feature-major ([d, token]); scores are computed
# transposed (S.T[key, q]) so the softmax reduction lands on the free
# axis of the AV matmul via an extra ones-row reduction, and the
# 1/denominator is applied after AV through a PE broadcast. Projections
# run as fp32r matmuls; the attention stage (scores/exp/AV) runs bf16.

import sys

sys.path.insert(0, "/opt/trn_rl_repo")

import numpy as np
import ml_dtypes

import concourse.bass as bass
import concourse.mybir as mybir
import concourse.tile as tile
from concourse import bacc

B, S, D, H, HD = 2, 2048, 512, 8, 64
WIN, DIL, G = 16, 2, 64
L = S - G  # 1984
NCORES = 8
TQ = 496  # local q tokens per core
QB = 84  # q block size
NBLK = 6  # blocks per core
TQP = QB * NBLK  # 504 padded q tokens
KW = QB + DIL * (WIN - 1) + 1  # 114 key window per block
PAD = DIL * (WIN // 2)  # 16 left halo
TKP = 544  # padded k/v tokens per core (16 + 496 + 32)
SCALE = 1.0 / np.sqrt(HD)
F32, F32R, BF16 = mybir.dt.float32, mybir.dt.float32r, mybir.dt.bfloat16
BF16_NP = ml_dtypes.bfloat16


def _r(ap):
    return ap  # all matmul operands are bf16


def _build():
    nc = bacc.Bacc("TRN2", target_bir_lowering=False, debug=False,
                   num_devices=NCORES)

    def din(name, shape, dt=F32):
        return nc.dram_tensor(name, shape, dt, kind="ExternalInput").ap()

    def dout(name, shape, dt=F32):
        return nc.dram_tensor(name, shape, dt, kind="ExternalOutput").ap()

    # local path inputs
    xq = din("xq", [4, 128, TQP], BF16)
    xk = din("xk", [4, 128, TKP], BF16)
    xv = din("xv", [4, 128, TKP], BF16)
    wq = din("wq", [4, 128, 512], BF16)
    wk = din("wk", [4, 128, 512], BF16)
    wv = din("wv", [4, 128, 512], BF16)
    wo = din("wo", [4, 128, 512], BF16)
    bq = din("bq", [128, 4])
    bk = din("bk", [128, 4])
    bo = din("bo", [128, 4])
    bv = din("bv", [1, 512], BF16)
    mask = din("mask", [KW, QB], BF16)
    ones_row = din("ones_row", [1, 128], BF16)
    ones_col = din("ones_col", [128, 1], BF16)
    ind_e = din("ind_e", [1, 128], BF16)
    ind_o = din("ind_o", [1, 128], BF16)
    # global path inputs (per-core head-pair slice)
    xgq = din("xgq", [4, 128, G], BF16)
    xgk = din("xgk", [4, 128, G], BF16)
    xgv = din("xgv", [4, 128, G], BF16)
    wgq = din("wgq", [4, 128, 128], BF16)
    wgk = din("wgk", [4, 128, 128], BF16)
    wgv = din("wgv", [4, 128, 128], BF16)
    gow = din("gow", [128, 512], BF16)
    bgq = din("bgq", [128, 1])
    bgk = din("bgk", [128, 1])
    bgv = din("bgv", [1, 128], BF16)
    id64 = din("id64", [64, 64])
    # outputs
    out_loc = dout("out_loc", [4, 128, TQ])
    out_g = dout("out_g", [4, 128, G])

    AF = mybir.ActivationFunctionType

    with tile.TileContext(nc) as tc:
        with tc.tile_pool(name="sb", bufs=1) as sb, \
             tc.tile_pool(name="ps", bufs=1, space="PSUM") as ps:

            def load(name, ap, shape=None, dt=F32):
                t = sb.tile(shape or list(ap.shape), dt, name=name)
                nc.sync.dma_start(t[:], ap[:])
                return t

            # ---- persistent SBUF: weights / inputs / constants ----
            wq_sb = sb.tile([128, 2048], BF16, name="wq_sb")
            wk_sb = sb.tile([128, 2048], BF16, name="wk_sb")
            wv_sb = sb.tile([128, 2048], BF16, name="wv_sb")
            wo_sb = sb.tile([128, 2048], BF16, name="wo_sb")
            xq_sb = sb.tile([128, 4 * TQP], BF16, name="xq_sb")
            xk_sb = sb.tile([128, 4 * TKP], BF16, name="xk_sb")
            xv_sb = sb.tile([128, 4 * TKP], BF16, name="xv_sb")
            for c in range(4):
                nc.sync.dma_start(wq_sb[:, 512 * c:512 * (c + 1)], wq[c])
                nc.sync.dma_start(wk_sb[:, 512 * c:512 * (c + 1)], wk[c])
                nc.sync.dma_start(wv_sb[:, 512 * c:512 * (c + 1)], wv[c])
                nc.sync.dma_start(wo_sb[:, 512 * c:512 * (c + 1)], wo[c])
                nc.sync.dma_start(xq_sb[:, TQP * c:TQP * (c + 1)], xq[c])
                nc.sync.dma_start(xk_sb[:, TKP * c:TKP * (c + 1)], xk[c])
                nc.sync.dma_start(xv_sb[:, TKP * c:TKP * (c + 1)], xv[c])
            bq_sb = load("bq_sb", bq)
            bk_sb = load("bk_sb", bk)
            bo_sb = load("bo_sb", bo)
            bv_sb = load("bv_sb", bv, dt=BF16)
            mask_sb = load("mask_sb", mask, dt=BF16)
            ones_r = load("ones_r", ones_row, dt=BF16)
            ones_c = load("ones_c", ones_col, dt=BF16)
            inde_sb = load("inde_sb", ind_e, dt=BF16)
            indo_sb = load("indo_sb", ind_o, dt=BF16)

            # ---- projections: q_f, k_f (feature-major, bf16 out) ----
            q_sb = sb.tile([128, 4 * TQP], BF16, name="q_sb")
            k_sb = sb.tile([128, 4 * TKP], BF16, name="k_sb")
            for c in range(4):
                qp = ps.tile([128, 512], F32, name="qp", tag="pj", bufs=2)
                for cc in range(4):
                    nc.tensor.matmul(
                        qp[:, :TQP],
                        _r(wq_sb[:, 512 * cc + 128 * c:512 * cc + 128 * (c + 1)]),
                        _r(xq_sb[:, TQP * cc:TQP * (cc + 1)]),
                        start=(cc == 0), stop=(cc == 3))
                nc.scalar.activation(q_sb[:, TQP * c:TQP * (c + 1)], qp[:, :TQP],
                                     AF.Identity, bias=bq_sb[:, c:c + 1])
                for half in range(2):
                    kp = ps.tile([128, 512], F32, name="kp", tag="pj", bufs=2)
                    hs = 272 * half
                    hn = TKP - 272 if half else 272
                    for cc in range(4):
                        nc.tensor.matmul(
                            kp[:, :hn],
                            _r(wk_sb[:, 512 * cc + 128 * c:512 * cc + 128 * (c + 1)]),
                            _r(xk_sb[:, TKP * cc + hs:TKP * cc + hs + hn]),
                            start=(cc == 0), stop=(cc == 3))
                    nc.scalar.activation(
                        k_sb[:, TKP * c + hs:TKP * c + hs + hn], kp[:, :hn],
                        AF.Identity, bias=bk_sb[:, c:c + 1])

            # ---- per-block: v projection (token-major) + attention ----
            o_sb = sb.tile([128, 4 * TQP], F32, name="o_sb")
            den_sb = sb.tile([1, 8 * TQP], BF16, name="den_sb")
            for b in range(NBLK):
                q0 = QB * b
                vbp = ps.tile([KW, 512], F32, name="vbp", tag="pj", bufs=2)
                for cc in range(4):
                    nc.tensor.matmul(
                        vbp[:, :],
                        _r(xv_sb[:, TKP * cc + q0:TKP * cc + q0 + KW]),
                        _r(wv_sb[:, 512 * cc:512 * (cc + 1)]),
                        start=(cc == 0), stop=False)
                nc.tensor.matmul(vbp[:, :], _r(ones_r[0:1, :KW]), _r(bv_sb[0:1, :]),
                                 start=False, stop=True)
                v_blk = sb.tile([KW, 512], BF16, name="v_blk", tag="vb", bufs=2)
                nc.vector.tensor_copy(v_blk[:, :], vbp[:, :])

                for hp in range(4):
                    avp = ps.tile([128, QB], F32, name="avp", tag="av", bufs=2, padded_shape=[128, 512])
                    dnp = ps.tile([1, 2 * QB], F32, name="dnp", tag="dn",
                                  bufs=2, padded_shape=[128, 512])
                    for hh in range(2):
                        h = 2 * hp + hh
                        r0 = 64 * hh
                        st = ps.tile([KW, QB], F32, name="st", tag="sc", bufs=2, padded_shape=[128, 512])
                        nc.tensor.matmul(
                            st[:, :],
                            k_sb[r0:r0 + 64, TKP * hp + q0:TKP * hp + q0 + KW],
                            q_sb[r0:r0 + 64, TQP * hp + q0:TQP * hp + q0 + QB],
                            start=True, stop=True)
                        es = sb.tile([KW, QB], BF16, name="es", tag="es", bufs=3)
                        nc.scalar.activation(es[:, :], st[:, :], AF.Exp, scale=SCALE)
                        em = sb.tile([KW, QB], BF16, name="em", tag="em", bufs=3)
                        nc.vector.tensor_mul(em[:, :], es[:, :], mask_sb[:, :])
                        nc.tensor.matmul(
                            avp[r0:r0 + 64, :],
                            v_blk[:, 64 * h:64 * (h + 1)], em[:, :],
                            start=True, stop=True)
                        nc.tensor.matmul(dnp[0:1, QB * hh:QB * (hh + 1)],
                                         ones_c[:KW, :], em[:, :],
                                         start=True, stop=True)
                    nc.vector.tensor_copy(o_sb[:, TQP * hp + q0:TQP * hp + q0 + QB],
                                          avp[:, :])
                    dst = den_sb[0:1, 2 * TQP * hp:2 * TQP * (hp + 1)]
                    dst = dst.rearrange("p (t q) -> p t q", t=2)
                    nc.vector.tensor_copy(
                        dst[:, :, q0:q0 + QB],
                        dnp[0:1, :].rearrange("p (t q) -> p t q", t=2))

            # ---- normalize (divide by broadcast denominator) + out-proj ----
            on_sb = sb.tile([128, 4 * TQP], BF16, name="on_sb")
            fin_sb = sb.tile([128, 4 * TQ], F32, name="fin_sb")
            for c in range(4):
                rp = ps.tile([128, 512], F32, name="rp", tag="pj", bufs=2)
                nc.tensor.matmul(rp[:, :TQP], inde_sb[0:1, :],
                                 den_sb[0:1, 2 * TQP * c:2 * TQP * c + TQP],
                                 start=True, stop=False)
                nc.tensor.matmul(rp[:, :TQP], indo_sb[0:1, :],
                                 den_sb[0:1,
                                        2 * TQP * c + TQP:2 * TQP * (c + 1)],
                                 start=False, stop=True)
                rcp = sb.tile([128, 512], F32, name="rcp", tag="rcp", bufs=2)
                nc.vector.reciprocal(rcp[:, :TQP], rp[:, :TQP])
                nc.vector.tensor_mul(on_sb[:, TQP * c:TQP * (c + 1)],
                                     o_sb[:, TQP * c:TQP * (c + 1)],
                                     rcp[:, :TQP])
            for c in range(4):
                op = ps.tile([128, 512], F32, name="op", tag="pj", bufs=2)
                for cc in range(4):
                    nc.tensor.matmul(
                        op[:, :TQP],
                        _r(wo_sb[:, 512 * cc + 128 * c:512 * cc + 128 * (c + 1)]),
                        _r(on_sb[:, TQP * cc:TQP * (cc + 1)]),
                        start=(cc == 0), stop=(cc == 3))
                nc.scalar.activation(fin_sb[:, TQ * c:TQ * (c + 1)], op[:, :TQ],
                                     AF.Identity, bias=bo_sb[:, c:c + 1])
                nc.sync.dma_start(out_loc[c], fin_sb[:, TQ * c:TQ * (c + 1)])

            # ---- global path (this core's 2 heads, full 64 tokens) ----
            wgq_sb = sb.tile([128, 512], BF16, name="wgq_sb")
            wgk_sb = sb.tile([128, 512], BF16, name="wgk_sb")
            wgv_sb = sb.tile([128, 512], BF16, name="wgv_sb")
            gow_sb = sb.tile([128, 512], BF16, name="gow_sb")
            xgq_sb = sb.tile([128, 4 * G], BF16, name="xgq_sb")
            xgk_sb = sb.tile([128, 4 * G], BF16, name="xgk_sb")
            xgv_sb = sb.tile([128, 4 * G], BF16, name="xgv_sb")
            for c in range(4):
                nc.sync.dma_start(wgq_sb[:, 128 * c:128 * (c + 1)], wgq[c])
                nc.sync.dma_start(wgk_sb[:, 128 * c:128 * (c + 1)], wgk[c])
                nc.sync.dma_start(wgv_sb[:, 128 * c:128 * (c + 1)], wgv[c])
                nc.sync.dma_start(xgq_sb[:, G * c:G * (c + 1)], xgq[c])
                nc.sync.dma_start(xgk_sb[:, G * c:G * (c + 1)], xgk[c])
                nc.sync.dma_start(xgv_sb[:, G * c:G * (c + 1)], xgv[c])
            nc.sync.dma_start(gow_sb[:, :], gow[:])
            bgq_sb = load("bgq_sb", bgq)
            bgk_sb = load("bgk_sb", bgk)
            bgv_sb = load("bgv_sb", bgv, dt=BF16)
            id_sb = load("id_sb", id64)

            qg_sb = sb.tile([128, G], BF16, name="qg_sb")
            kg_sb = sb.tile([128, G], BF16, name="kg_sb")
            vg_sb = sb.tile([G, 128], BF16, name="vg_sb")
            gq = ps.tile([128, G], F32, name="gq", tag="av", bufs=2, padded_shape=[128, 512])
            for cc in range(4):
                nc.tensor.matmul(gq[:, :], _r(wgq_sb[:, 128 * cc:128 * (cc + 1)]),
                                 _r(xgq_sb[:, G * cc:G * (cc + 1)]),
                                 start=(cc == 0), stop=(cc == 3))
            nc.scalar.activation(qg_sb[:, :], gq[:, :], AF.Identity,
                                 bias=bgq_sb[:, 0:1])
            gk = ps.tile([128, G], F32, name="gk", tag="av", bufs=2, padded_shape=[128, 512])
            for cc in range(4):
                nc.tensor.matmul(gk[:, :], _r(wgk_sb[:, 128 * cc:128 * (cc + 1)]),
                                 _r(xgk_sb[:, G * cc:G * (cc + 1)]),
                                 start=(cc == 0), stop=(cc == 3))
            nc.scalar.activation(kg_sb[:, :], gk[:, :], AF.Identity,
                                 bias=bgk_sb[:, 0:1])
            gv = ps.tile([G, 128], F32, name="gv", tag="av", bufs=2, padded_shape=[128, 512])
            for cc in range(4):
                nc.tensor.matmul(gv[:, :], _r(xgv_sb[:, G * cc:G * (cc + 1)]),
                                 _r(wgv_sb[:, 128 * cc:128 * (cc + 1)]),
                                 start=(cc == 0), stop=False)
            nc.tensor.matmul(gv[:, :], _r(ones_r[0:1, :G]), _r(bgv_sb[0:1, :]),
                             start=False, stop=True)
            nc.vector.tensor_copy(vg_sb[:, :], gv[:, :])

            og = ps.tile([128, G], F32, name="og", tag="av", bufs=2, padded_shape=[128, 512])
            for hh in range(2):
                r0 = 64 * hh
                sg = ps.tile([64, 64], F32, name="sg", tag="sc", bufs=2, padded_shape=[128, 512])
                nc.tensor.matmul(sg[:, :], qg_sb[r0:r0 + 64, :],
                                 kg_sb[r0:r0 + 64, :], start=True, stop=True)
                pg = sb.tile([64, 64], F32, name="pg", tag="pg", bufs=2)
                dg = sb.tile([64, 1], F32, name="dg", tag="dg", bufs=2)
                nc.scalar.activation(pg[:, :], sg[:, :], AF.Exp, scale=SCALE,
                                     accum_out=dg[:, :])
                rg = sb.tile([64, 1], F32, name="rg", tag="rg", bufs=2)
                nc.vector.reciprocal(rg[:, :], dg[:, :])
                pn = sb.tile([64, 64], F32, name="pn", tag="pn", bufs=2)
                nc.vector.tensor_scalar_mul(pn[:, :], pg[:, :], rg[:, :])
                tp = ps.tile([64, 64], F32, name="tp", tag="sc", bufs=2, padded_shape=[128, 512])
                nc.tensor.transpose(tp[:, :], pn[:, :], id_sb[:, :])
                pt = sb.tile([64, 64], BF16, name="pt", tag="pt", bufs=2)
                nc.vector.tensor_copy(pt[:, :], tp[:, :])
                nc.tensor.matmul(og[r0:r0 + 64, :], vg_sb[:, r0:r0 + 64],
                                 pt[:, :], start=True, stop=True)
            og_sb = sb.tile([128, G], BF16, name="og_sb")
            nc.vector.tensor_copy(og_sb[:, :], og[:, :])
            gp_sb = sb.tile([128, 4 * G], F32, name="gp_sb")
            for c in range(4):
                go = ps.tile([128, G], F32, name="go", tag="av", bufs=2, padded_shape=[128, 512])
                nc.tensor.matmul(go[:, :], _r(gow_sb[:, 128 * c:128 * (c + 1)]),
                                 _r(og_sb[:, :]), start=True, stop=True)
                nc.vector.tensor_copy(gp_sb[:, G * c:G * (c + 1)], go[:, :])
                nc.sync.dma_start(out_g[c], gp_sb[:, G * c:G * (c + 1)])

    nc.compile()
    return nc


_CACHE = {}


def _get_nc():
    if "nc" not in _CACHE:
        _CACHE["nc"] = _build()
    return _CACHE["nc"]


def _chunk4(a):
    # [512, N] -> [4, 128, N]
    return np.ascontiguousarray(a.reshape(4, 128, -1))


def make_core_inputs(query, key, value, wq, bq, wk, bk, wv, bv, wo, bo,
                     g_in_w, g_in_b, g_out_w, g_out_b):
    """Build the 8 per-core input dicts (host-side sharding)."""
    f32 = np.float32
    wq_t = _chunk4(np.ascontiguousarray(wq.T).astype(BF16_NP))
    wk_t = _chunk4(np.ascontiguousarray(wk.T).astype(BF16_NP))
    wv_t = _chunk4(np.ascontiguousarray(wv.T).astype(BF16_NP))
    wo_t = _chunk4(np.ascontiguousarray(wo.T).astype(BF16_NP))
    bq_r = np.ascontiguousarray(bq.reshape(4, 128).T).astype(f32)
    bk_r = np.ascontiguousarray(bk.reshape(4, 128).T).astype(f32)
    bo_r = np.ascontiguousarray(bo.reshape(4, 128).T).astype(f32)
    bv_r = bv.reshape(1, 512).astype(BF16_NP)

    jk = np.arange(KW)[:, None]
    p = np.arange(QB)[None, :]
    d = jk - p
    mask = (((d >= 0) & (d <= DIL * (WIN - 1)) & (d % 2 == 0))
            .astype(BF16_NP))
    ones_row = np.ones((1, 128), BF16_NP)
    ones_col = np.ones((128, 1), BF16_NP)
    ind_e = np.zeros((1, 128), BF16_NP)
    ind_e[0, :64] = 1.0
    ind_o = np.zeros((1, 128), BF16_NP)
    ind_o[0, 64:] = 1.0
    id64 = np.eye(64, dtype=f32)

    wq_g, wk_g, wv_g = g_in_w[:D], g_in_w[D:2 * D], g_in_w[2 * D:]
    bq_g, bk_g, bv_g = g_in_b[:D], g_in_b[D:2 * D], g_in_b[2 * D:]

    in_maps = []
    for c in range(NCORES):
        b, j = c // 4, c % 4
        q0 = TQ * j
        # q tokens, zero-padded to TQP, feature-major
        xq_t = np.zeros((512, TQP), BF16_NP)
        xq_t[:, :TQ] = query[b, G + q0:G + q0 + TQ, :].T.astype(BF16_NP)
        # k/v halo slice with edge-replication padding
        kidx = np.clip(q0 - PAD + np.arange(TKP), 0, L - 1)
        xk_t = np.ascontiguousarray(key[b, G:, :][kidx].T).astype(BF16_NP)
        xv_t = np.ascontiguousarray(value[b, G:, :][kidx].T).astype(BF16_NP)
        # global slice: head-pair j
        hs = slice(128 * j, 128 * (j + 1))
        m = {
            "xq": _chunk4(xq_t), "xk": _chunk4(xk_t), "xv": _chunk4(xv_t),
            "wq": wq_t, "wk": wk_t, "wv": wv_t, "wo": wo_t,
            "bq": bq_r, "bk": bk_r, "bo": bo_r, "bv": bv_r,
            "mask": mask, "ones_row": ones_row, "ones_col": ones_col,
            "ind_e": ind_e, "ind_o": ind_o,
            "xgq": _chunk4(np.ascontiguousarray(query[b, :G, :].T).astype(BF16_NP)),
            "xgk": _chunk4(np.ascontiguousarray(key[b, :G, :].T).astype(BF16_NP)),
            "xgv": _chunk4(np.ascontiguousarray(value[b, :G, :].T).astype(BF16_NP)),
            "wgq": _chunk4(np.ascontiguousarray(wq_g[hs].T).astype(BF16_NP)),
            "wgk": _chunk4(np.ascontiguousarray(wk_g[hs].T).astype(BF16_NP)),
            "wgv": _chunk4(np.ascontiguousarray(wv_g[hs].T).astype(BF16_NP)),
            "gow": np.ascontiguousarray(g_out_w[:, hs].T).astype(BF16_NP),
            "bgq": bq_g[hs][:, None].astype(f32),
            "bgk": bk_g[hs][:, None].astype(f32),
            "bgv": bv_g[hs][None, :].astype(BF16_NP),
            "id64": id64,
        }
        in_maps.append(m)
    return in_maps


def assemble(results, g_out_b):
    out = np.zeros((B, S, D), np.float32)
    for c in range(NCORES):
        b, j = c // 4, c % 4
        ol = results[c]["out_loc"].reshape(512, TQ)
        out[b, G + TQ * j:G + TQ * (j + 1), :] = ol.T
    for b in range(B):
        acc = np.zeros((512, G), np.float32)
        for j in range(4):
            acc += results[b * 4 + j]["out_g"].reshape(512, G)
        out[b, :G, :] = acc.T + g_out_b[None, :].astype(np.float32)
    return out


def kernel(**inputs):
    from concourse import bass_utils
    nc = _get_nc()
    in_maps = make_core_inputs(**{k: np.asarray(v) for k, v in inputs.items()})
    res = bass_utils.run_bass_kernel_spmd(nc, in_maps,
                                          core_ids=list(range(NCORES)))
    return assemble(res.results, np.asarray(inputs["g_out_b"]))
